# revision 17
# baseline (speedup 1.0000x reference)
"""Single-head attention (B=8, S=2048, D=1024, d_k=512), data-parallel over
batch across 8 NeuronCores, all matmuls in fp16 (1 cycle/row on the PE).

Per-core dataflow (batch element b on core b):

  host:  xT16 = x[b].T in f16                         [1024, 2048]
  Q^T = Wq^T x + bq  -> [dk, S]  (k on partitions)    lhsT=Wq tile, rhs=xT
  K^T = Wk^T x + bk  -> [dk, S]
  V   = x^T Wv + bv  -> [S, dk]  (s on partitions)    bv via DVE broadcast add
  S^T[s,q] tiles: lhsT=K^T tile, rhs=Q^T chunk
  E = exp(S^T / sqrt(dk)) -> f16  (|scores| < ~4, no max subtraction)
  acc_z = DVE running sum of E tiles (fp32), Z = ones^T @ acc_z16  [1, q]
  zcol[q] via transpose-matmul (z16 slice as weights x [1,1] one)
  1/Z = DVE reciprocal on [128,1] columns
  out[q,k] = PV: lhsT=E tile [s-part, q-cols], rhs=V [s-part, dk]
  eviction: ACT copy with per-partition scale 1/Z  (bv self-normalizes
  since sum(a)=1 and bv was folded into V)
  out[b] is written in natural [S, dk] layout - host just stacks.

All SBUF/PSUM pools are opened once at the top (PSUM shared between the
projection and attention phases) so no mid-kernel pool-transition barriers
are emitted. First-chunk DMAs are spread over 4 DGE rings.
"""

import numpy as np

import concourse.bass as bass
import concourse.mybir as mybir
import concourse.tile as tile

B, S, D, DK = 8, 2048, 1024, 512
N_CORES = 8
P = 128
DT = D // P      # 8 d-tiles (contraction tiles for projections)
MT = DK // P     # 4 k-tiles
ST = S // P      # 16 s-tiles
NCH = S // 512   # 4 free-dim chunks of 512
SCALE = float(1.0 / np.sqrt(np.float32(DK)))

F32 = mybir.dt.float32
F16 = mybir.dt.float16


def _split_excess_waits(nc, max_waits=1):
    """This walrus build accepts very few sync waits per instruction (and adds
    its own implicit queue waits to Drain). Move excess BIR waits onto
    dedicated NoOps inserted just before the over-subscribed instruction."""
    count = 0
    for f in nc.m.functions:
        for b in f.blocks:
            insts = list(b.instructions)
            out = []
            for ins in insts:
                si = getattr(ins, "sync_info", None)
                waits = list(si.on_wait) if si is not None else []
                cap = 0 if isinstance(ins, mybir.InstDrain) else max_waits
                if len(waits) > cap:
                    keep = waits[len(waits) - cap:] if cap else []
                    excess = waits[: len(waits) - cap]
                    for i in range(0, len(excess), max_waits):
                        chunk = excess[i : i + max_waits]
                        count += 1
                        nop = mybir.InstNoOp(
                            name=f"Wsplit-{count}", engine=ins.engine
                        )
                        nop.sync_info = mybir.SyncInfo(
                            on_wait=chunk, on_update=[]
                        )
                        out.append(nop)
                    ins.sync_info = mybir.SyncInfo(
                        on_wait=keep, on_update=list(si.on_update)
                    )
                out.append(ins)
            live = b.instructions
            live.clear()
            live.extend(out)
    return count


def build_nc(split_waits=True):
    nc = bass.Bass()
    xT = nc.dram_tensor("xT", [D, S], F16, kind="ExternalInput")
    wq = nc.dram_tensor("wq", [D, DK], F16, kind="ExternalInput")
    wk = nc.dram_tensor("wk", [D, DK], F16, kind="ExternalInput")
    wv = nc.dram_tensor("wv", [D, DK], F16, kind="ExternalInput")
    bq = nc.dram_tensor("bq", [P, MT], F32, kind="ExternalInput")
    bk = nc.dram_tensor("bk", [P, MT], F32, kind="ExternalInput")
    bvb = nc.dram_tensor("bvb", [P, DK], F16, kind="ExternalInput")
    ones_c = nc.dram_tensor("ones_c", [P, 1], F16, kind="ExternalInput")
    one1 = nc.dram_tensor("one1", [1, 1], F16, kind="ExternalInput")
    out = nc.dram_tensor("out", [S, DK], F32, kind="ExternalOutput")

    xT_r = xT.rearrange("(dt p) s -> p dt s", p=P)
    wq_r = wq.rearrange("(dt p) k -> p dt k", p=P)
    wk_r = wk.rearrange("(dt p) k -> p dt k", p=P)
    wv_r = wv.rearrange("(dt p) k -> p dt k", p=P)

    with tile.TileContext(nc) as tc:
        with tc.tile_pool(name="persist", bufs=1) as persist, \
             tc.tile_pool(name="wpool", bufs=1) as wpool, \
             tc.tile_pool(name="xpool", bufs=2) as xpool, \
             tc.tile_pool(name="epool", bufs=2) as epool, \
             tc.tile_pool(name="spool", bufs=2) as spool, \
             tc.tile_pool(name="opool", bufs=3) as opool, \
             tc.tile_pool(name="psA", bufs=3, space="PSUM") as psA, \
             tc.tile_pool(name="psO", bufs=2, space="PSUM") as psO, \
             tc.tile_pool(name="psZ", bufs=1, space="PSUM") as psZ, \
             tc.tile_pool(name="psY", bufs=2, space="PSUM") as psY:
            qT = persist.tile([P, MT, S], F16, tag="qT")
            kT = persist.tile([P, MT, S], F16, tag="kT")
            v_sb = persist.tile([P, ST, DK], F16, tag="v")
            bq_sb = persist.tile([P, MT], F32, tag="bq")
            bk_sb = persist.tile([P, MT], F32, tag="bk")
            bv_sb = persist.tile([P, DK], F16, tag="bv")
            ones_col = persist.tile([P, 1], F16, tag="ones_col")
            one_one = persist.tile([1, 1], F16, tag="one_one")

            wq_sb = [wpool.tile([P, DK], F16, tag=f"wq{d}", name=f"wq{d}") for d in range(DT)]
            wk_sb = [wpool.tile([P, DK], F16, tag=f"wk{d}", name=f"wk{d}") for d in range(DT)]
            wv_sb = [wpool.tile([P, DK], F16, tag=f"wv{d}", name=f"wv{d}") for d in range(DT)]
            xt0 = [xpool.tile([P, 512], F16, tag=f"xt{d}", name=f"xt{d}") for d in range(DT)]

            # ---------- input DMAs across the 3 DGE rings ----------
            # sync:   xt0 d0-5 | wv d0-3 | xt1.. halves
            # scalar: wq d0-5  | wk d0-5 | bv
            # gpsimd: xt0 d6-7, wq d6-7 | consts | wk d6-7 | wv d4-7
            for d in range(6):
                nc.sync.dma_start(out=xt0[d], in_=xT_r[:, d, 0:512])
                nc.scalar.dma_start(out=wq_sb[d], in_=wq_r[:, d, :])
            for d in (6, 7):
                nc.gpsimd.dma_start(out=xt0[d], in_=xT_r[:, d, 0:512])
                nc.gpsimd.dma_start(out=wq_sb[d], in_=wq_r[:, d, :])
            nc.gpsimd.dma_start(out=bq_sb, in_=bq[:, :])
            nc.gpsimd.dma_start(out=bk_sb, in_=bk[:, :])
            nc.gpsimd.dma_start(out=ones_col, in_=ones_c[:, :])
            nc.gpsimd.dma_start(out=one_one, in_=one1[:, :])
            for d in range(6):
                nc.scalar.dma_start(out=wk_sb[d], in_=wk_r[:, d, :])
            for d in (6, 7):
                nc.gpsimd.dma_start(out=wk_sb[d], in_=wk_r[:, d, :])
            for d in range(4):
                nc.sync.dma_start(out=wv_sb[d], in_=wv_r[:, d, :])
            for d in range(4, DT):
                nc.gpsimd.dma_start(out=wv_sb[d], in_=wv_r[:, d, :])
            nc.scalar.dma_start(out=bv_sb, in_=bvb[:, :])

            # ---------- Phase B: projections ----------
            for sc in range(NCH):
                if sc == 0:
                    xt = xt0
                else:
                    xt = [
                        xpool.tile([P, 512], F16, tag=f"xt{d}", name=f"xt{d}")
                        for d in range(DT)
                    ]
                    for d in range(4):
                        nc.sync.dma_start(
                            out=xt[d],
                            in_=xT_r[:, d, sc * 512 : (sc + 1) * 512],
                        )
                    for d in range(4, DT):
                        nc.scalar.dma_start(
                            out=xt[d],
                            in_=xT_r[:, d, sc * 512 : (sc + 1) * 512],
                        )
                # Q^T then K^T chunks: [k-part, 512 s]
                for m in range(MT):
                    psq = psA.tile([P, 512], F32, tag="ps")
                    for d in range(DT):
                        nc.tensor.matmul(
                            psq,
                            lhsT=wq_sb[d][:, m * P : (m + 1) * P],
                            rhs=xt[d],
                            start=(d == 0),
                            stop=(d == DT - 1),
                        )
                    nc.vector.tensor_scalar_add(
                        qT[:, m, sc * 512 : (sc + 1) * 512],
                        psq,
                        bq_sb[:, m : m + 1],
                    )
                for m in range(MT):
                    psk = psA.tile([P, 512], F32, tag="ps")
                    for d in range(DT):
                        nc.tensor.matmul(
                            psk,
                            lhsT=wk_sb[d][:, m * P : (m + 1) * P],
                            rhs=xt[d],
                            start=(d == 0),
                            stop=(d == DT - 1),
                        )
                    nc.vector.tensor_scalar_add(
                        kT[:, m, sc * 512 : (sc + 1) * 512],
                        psk,
                        bk_sb[:, m : m + 1],
                    )
                # V rows for this s-chunk: [s-part, dk]; bv via DVE add
                for i in range(4):
                    st = sc * 4 + i
                    psv = psA.tile([P, 512], F32, tag="ps")
                    for d in range(DT):
                        nc.tensor.matmul(
                            psv,
                            lhsT=xt[d][:, i * P : (i + 1) * P],
                            rhs=wv_sb[d],
                            start=(d == 0),
                            stop=(d == DT - 1),
                        )
                    nc.vector.tensor_add(v_sb[:, st, :], psv, bv_sb)

            # ---------- Phase C: attention ----------
            for qc in range(NCH):
                eT = epool.tile([P, ST, 512], F16, tag="eT")
                acc_z = spool.tile([P, 512], F32, tag="acc_z")
                # S^T tiles: [s-part, 512 q], exp on eviction
                for st in range(ST):
                    pss = psA.tile([P, 512], F32, tag="ps")
                    for kt in range(MT):
                        nc.tensor.matmul(
                            pss,
                            lhsT=kT[:, kt, st * P : (st + 1) * P],
                            rhs=qT[:, kt, qc * 512 : (qc + 1) * 512],
                            start=(kt == 0),
                            stop=(kt == MT - 1),
                        )
                    nc.scalar.activation(
                        out=eT[:, st, :],
                        in_=pss,
                        func=mybir.ActivationFunctionType.Exp,
                        scale=SCALE,
                    )
                    if st == 0:
                        nc.vector.tensor_copy(acc_z, eT[:, 0, :])
                    else:
                        nc.vector.tensor_add(acc_z, acc_z, eT[:, st, :])
                # Z = ones^T @ acc_z16 -> [1, 512], transpose to columns
                # via tiny matmuls, reciprocal per 128-block. The z-chain PE
                # work is emitted after the first PV group so the scalar /
                # vector softmax-sum chain never stalls the PE.
                acc_z16 = spool.tile([P, 512], F16, tag="acc_z16")
                nc.scalar.copy(acc_z16, acc_z)
                zcol = spool.tile([P, MT], F32, tag="zcol")
                # PV: out[q, k] tiles, q on partitions
                for j in range(MT):
                    pso = psO.tile([P, 512], F32, tag="pso")
                    for st in range(ST):
                        nc.tensor.matmul(
                            pso,
                            lhsT=eT[:, st, j * P : (j + 1) * P],
                            rhs=v_sb[:, st, :],
                            start=(st == 0),
                            stop=(st == ST - 1),
                        )
                    if j == 0:
                        psz = psZ.tile([1, 512], F32, tag="psz")
                        nc.tensor.matmul(
                            psz,
                            lhsT=ones_col[:, 0:1],
                            rhs=acc_z16,
                            start=True,
                            stop=True,
                        )
                        z16 = spool.tile([1, 512], F16, tag="z16")
                        nc.scalar.copy(z16[0:1, :], psz[0:1, :])
                        for jj in range(MT):
                            zc = psY.tile([P, 1], F32, tag="zc")
                            nc.tensor.matmul(
                                zc,
                                lhsT=z16[0:1, jj * P : (jj + 1) * P],
                                rhs=one_one[0:1, 0:1],
                                start=True,
                                stop=True,
                            )
                            nc.vector.reciprocal(zcol[:, jj : jj + 1], zc)
                    o = opool.tile([P, 512], F32, tag="o")
                    nc.scalar.activation(
                        out=o,
                        in_=pso,
                        func=mybir.ActivationFunctionType.Copy,
                        scale=zcol[:, j : j + 1],
                    )
                    ring = nc.sync if j % 2 == 0 else nc.scalar
                    ring.dma_start(
                        out=out[qc * 512 + j * P : qc * 512 + (j + 1) * P, :],
                        in_=o,
                    )

    if split_waits:
        _split_excess_waits(nc)
    return nc


_NC_CACHE = None


def _get_nc():
    global _NC_CACHE
    if _NC_CACHE is None:
        _NC_CACHE = build_nc()
    return _NC_CACHE


def _make_in_maps(x, Wq, bq, Wk, bk, Wv, bv):
    x = np.asarray(x, dtype=np.float32)
    wq16 = np.ascontiguousarray(np.asarray(Wq, np.float32).astype(np.float16))
    wk16 = np.ascontiguousarray(np.asarray(Wk, np.float32).astype(np.float16))
    wv16 = np.ascontiguousarray(np.asarray(Wv, np.float32).astype(np.float16))
    bq_c = np.ascontiguousarray(np.asarray(bq, np.float32).reshape(MT, P).T)
    bk_c = np.ascontiguousarray(np.asarray(bk, np.float32).reshape(MT, P).T)
    bv_b = np.ascontiguousarray(
        np.broadcast_to(
            np.asarray(bv, np.float32).astype(np.float16).reshape(1, DK),
            (P, DK),
        )
    )
    ones_c = np.ones((P, 1), dtype=np.float16)
    one1 = np.ones((1, 1), dtype=np.float16)
    in_maps = []
    for c in range(N_CORES):
        in_maps.append(
            {
                "xT": np.ascontiguousarray(x[c].T.astype(np.float16)),
                "wq": wq16,
                "wk": wk16,
                "wv": wv16,
                "bq": bq_c,
                "bk": bk_c,
                "bvb": bv_b,
                "ones_c": ones_c,
                "one1": one1,
            }
        )
    return in_maps


def run(x, Wq, bq, Wk, bk, Wv, bv, **run_kwargs):
    """Run on the 8 NeuronCores; returns (output, BassKernelResults)."""
    from concourse.bass_utils import run_bass_kernel_spmd

    nc = _get_nc()
    in_maps = _make_in_maps(x, Wq, bq, Wk, bk, Wv, bv)
    res = run_bass_kernel_spmd(
        nc, in_maps, core_ids=list(range(N_CORES)), **run_kwargs
    )
    out = np.stack([r["out"] for r in res.results], axis=0)
    return out, res


def kernel(x, Wq, bq, Wk, bk, Wv, bv):
    out, _ = run(x, Wq, bq, Wk, bk, Wv, bv)
    return out


# revision 18
# speedup vs baseline: 1.0036x; 1.0036x over previous
"""Single-head attention (B=8, S=2048, D=1024, d_k=512), data-parallel over
batch across 8 NeuronCores, all matmuls in fp16 (1 cycle/row on the PE).

Per-core dataflow (batch element b on core b):

  host:  xT16 = x[b].T in f16                         [1024, 2048]
  Q^T = Wq^T x + bq  -> [dk, S]  (k on partitions)    lhsT=Wq tile, rhs=xT
  K^T = Wk^T x + bk  -> [dk, S]
  V   = x^T Wv + bv  -> [S, dk]  (s on partitions)    bv via DVE broadcast add
  S^T[s,q] tiles: lhsT=K^T tile, rhs=Q^T chunk
  E = exp(S^T / sqrt(dk)) -> f16  (|scores| < ~4, no max subtraction)
  acc_z = DVE running sum of E tiles (fp32), Z = ones^T @ acc_z16  [1, q]
  zcol[q] via transpose-matmul (z16 slice as weights x [1,1] one)
  1/Z = DVE reciprocal on [128,1] columns
  out[q,k] = PV: lhsT=E tile [s-part, q-cols], rhs=V [s-part, dk]
  eviction: ACT copy with per-partition scale 1/Z  (bv self-normalizes
  since sum(a)=1 and bv was folded into V)
  out[b] is written in natural [S, dk] layout - host just stacks.

All SBUF/PSUM pools are opened once at the top (PSUM shared between the
projection and attention phases) so no mid-kernel pool-transition barriers
are emitted. First-chunk DMAs are spread over 4 DGE rings.
"""

import numpy as np

import concourse.bass as bass
import concourse.mybir as mybir
import concourse.tile as tile

B, S, D, DK = 8, 2048, 1024, 512
N_CORES = 8
P = 128
DT = D // P      # 8 d-tiles (contraction tiles for projections)
MT = DK // P     # 4 k-tiles
ST = S // P      # 16 s-tiles
NCH = S // 512   # 4 free-dim chunks of 512
SCALE = float(1.0 / np.sqrt(np.float32(DK)))

F32 = mybir.dt.float32
F16 = mybir.dt.float16


def _split_excess_waits(nc, max_waits=1):
    """This walrus build accepts very few sync waits per instruction (and adds
    its own implicit queue waits to Drain). Move excess BIR waits onto
    dedicated NoOps inserted just before the over-subscribed instruction."""
    count = 0
    for f in nc.m.functions:
        for b in f.blocks:
            insts = list(b.instructions)
            out = []
            for ins in insts:
                si = getattr(ins, "sync_info", None)
                waits = list(si.on_wait) if si is not None else []
                cap = 0 if isinstance(ins, mybir.InstDrain) else max_waits
                if len(waits) > cap:
                    keep = waits[len(waits) - cap:] if cap else []
                    excess = waits[: len(waits) - cap]
                    for i in range(0, len(excess), max_waits):
                        chunk = excess[i : i + max_waits]
                        count += 1
                        nop = mybir.InstNoOp(
                            name=f"Wsplit-{count}", engine=ins.engine
                        )
                        nop.sync_info = mybir.SyncInfo(
                            on_wait=chunk, on_update=[]
                        )
                        out.append(nop)
                    ins.sync_info = mybir.SyncInfo(
                        on_wait=keep, on_update=list(si.on_update)
                    )
                out.append(ins)
            live = b.instructions
            live.clear()
            live.extend(out)
    return count


def build_nc(split_waits=True):
    nc = bass.Bass()
    xT = nc.dram_tensor("xT", [D, S], F16, kind="ExternalInput")
    wq = nc.dram_tensor("wq", [D, DK], F16, kind="ExternalInput")
    wk = nc.dram_tensor("wk", [D, DK], F16, kind="ExternalInput")
    wv = nc.dram_tensor("wv", [D, DK], F16, kind="ExternalInput")
    bq = nc.dram_tensor("bq", [P, MT], F32, kind="ExternalInput")
    bk = nc.dram_tensor("bk", [P, MT], F32, kind="ExternalInput")
    bvb = nc.dram_tensor("bvb", [P, DK], F16, kind="ExternalInput")
    ones_c = nc.dram_tensor("ones_c", [P, 1], F16, kind="ExternalInput")
    one1 = nc.dram_tensor("one1", [1, 1], F16, kind="ExternalInput")
    out = nc.dram_tensor("out", [S, DK], F32, kind="ExternalOutput")

    xT_r = xT.rearrange("(dt p) s -> p dt s", p=P)
    wq_r = wq.rearrange("(dt p) k -> p dt k", p=P)
    wk_r = wk.rearrange("(dt p) k -> p dt k", p=P)
    wv_r = wv.rearrange("(dt p) k -> p dt k", p=P)

    with tile.TileContext(nc) as tc:
        with tc.tile_pool(name="persist", bufs=1) as persist, \
             tc.tile_pool(name="wpool", bufs=1) as wpool, \
             tc.tile_pool(name="xpool", bufs=2) as xpool, \
             tc.tile_pool(name="epool", bufs=2) as epool, \
             tc.tile_pool(name="spool", bufs=2) as spool, \
             tc.tile_pool(name="opool", bufs=3) as opool, \
             tc.tile_pool(name="psA", bufs=3, space="PSUM") as psA, \
             tc.tile_pool(name="psO", bufs=2, space="PSUM") as psO, \
             tc.tile_pool(name="psZ", bufs=1, space="PSUM") as psZ, \
             tc.tile_pool(name="psY", bufs=2, space="PSUM") as psY:
            qT = persist.tile([P, MT, S], F16, tag="qT")
            kT = persist.tile([P, MT, S], F16, tag="kT")
            v_sb = persist.tile([P, ST, DK], F16, tag="v")
            bq_sb = persist.tile([P, MT], F32, tag="bq")
            bk_sb = persist.tile([P, MT], F32, tag="bk")
            bv_sb = persist.tile([P, DK], F16, tag="bv")
            ones_col = persist.tile([P, 1], F16, tag="ones_col")
            one_one = persist.tile([1, 1], F16, tag="one_one")

            wq_sb = [wpool.tile([P, DK], F16, tag=f"wq{d}", name=f"wq{d}") for d in range(DT)]
            wk_sb = [wpool.tile([P, DK], F16, tag=f"wk{d}", name=f"wk{d}") for d in range(DT)]
            wv_sb = [wpool.tile([P, DK], F16, tag=f"wv{d}", name=f"wv{d}") for d in range(DT)]
            xt0 = [xpool.tile([P, 512], F16, tag=f"xt{d}", name=f"xt{d}") for d in range(DT)]

            # ---------- input DMAs across the 3 DGE rings ----------
            # sync:   xt0 d0-5 | wv d0-3 | xt1.. halves
            # scalar: wq d0-5  | wk d0-5 | bv
            # gpsimd: xt0 d6-7, wq d6-7 | consts | wk d6-7 | wv d4-7
            for d in range(6):
                nc.sync.dma_start(out=xt0[d], in_=xT_r[:, d, 0:512])
                nc.scalar.dma_start(out=wq_sb[d], in_=wq_r[:, d, :])
            for d in (6, 7):
                nc.gpsimd.dma_start(out=xt0[d], in_=xT_r[:, d, 0:512])
                nc.gpsimd.dma_start(out=wq_sb[d], in_=wq_r[:, d, :])
            nc.gpsimd.dma_start(out=bq_sb, in_=bq[:, :])
            nc.gpsimd.dma_start(out=bk_sb, in_=bk[:, :])
            nc.gpsimd.dma_start(out=ones_col, in_=ones_c[:, :])
            nc.gpsimd.dma_start(out=one_one, in_=one1[:, :])
            for d in range(6):
                nc.scalar.dma_start(out=wk_sb[d], in_=wk_r[:, d, :])
            for d in (6, 7):
                nc.gpsimd.dma_start(out=wk_sb[d], in_=wk_r[:, d, :])
            for d in range(4):
                nc.sync.dma_start(out=wv_sb[d], in_=wv_r[:, d, :])
            for d in range(4, DT):
                nc.gpsimd.dma_start(out=wv_sb[d], in_=wv_r[:, d, :])
            nc.scalar.dma_start(out=bv_sb, in_=bvb[:, :])

            # ---------- Phase B: projections ----------
            for sc in range(NCH):
                if sc == 0:
                    xt = xt0
                else:
                    xt = [
                        xpool.tile([P, 512], F16, tag=f"xt{d}", name=f"xt{d}")
                        for d in range(DT)
                    ]
                    for d in range(4):
                        nc.sync.dma_start(
                            out=xt[d],
                            in_=xT_r[:, d, sc * 512 : (sc + 1) * 512],
                        )
                    for d in range(4, DT):
                        nc.scalar.dma_start(
                            out=xt[d],
                            in_=xT_r[:, d, sc * 512 : (sc + 1) * 512],
                        )
                # Q^T then K^T chunks: [k-part, 512 s]
                for m in range(MT):
                    psq = psA.tile([P, 512], F32, tag="ps")
                    for d in range(DT):
                        nc.tensor.matmul(
                            psq,
                            lhsT=wq_sb[d][:, m * P : (m + 1) * P],
                            rhs=xt[d],
                            start=(d == 0),
                            stop=(d == DT - 1),
                        )
                    nc.vector.tensor_scalar_add(
                        qT[:, m, sc * 512 : (sc + 1) * 512],
                        psq,
                        bq_sb[:, m : m + 1],
                    )
                for m in range(MT):
                    psk = psA.tile([P, 512], F32, tag="ps")
                    for d in range(DT):
                        nc.tensor.matmul(
                            psk,
                            lhsT=wk_sb[d][:, m * P : (m + 1) * P],
                            rhs=xt[d],
                            start=(d == 0),
                            stop=(d == DT - 1),
                        )
                    nc.vector.tensor_scalar_add(
                        kT[:, m, sc * 512 : (sc + 1) * 512],
                        psk,
                        bk_sb[:, m : m + 1],
                    )
                # V rows for this s-chunk: [s-part, dk]; bv via DVE add
                for i in range(4):
                    st = sc * 4 + i
                    psv = psA.tile([P, 512], F32, tag="ps")
                    for d in range(DT):
                        nc.tensor.matmul(
                            psv,
                            lhsT=xt[d][:, i * P : (i + 1) * P],
                            rhs=wv_sb[d],
                            start=(d == 0),
                            stop=(d == DT - 1),
                        )
                    nc.vector.tensor_add(v_sb[:, st, :], psv, bv_sb)

            # ---------- Phase C: attention ----------
            for qc in range(NCH):
                eT = epool.tile([P, ST, 512], F16, tag="eT")
                acc_z = spool.tile([P, 512], F32, tag="acc_z")
                # S^T tiles: [s-part, 512 q], exp on eviction
                for st in range(ST):
                    pss = psA.tile([P, 512], F32, tag="ps")
                    for kt in range(MT):
                        nc.tensor.matmul(
                            pss,
                            lhsT=kT[:, kt, st * P : (st + 1) * P],
                            rhs=qT[:, kt, qc * 512 : (qc + 1) * 512],
                            start=(kt == 0),
                            stop=(kt == MT - 1),
                        )
                    nc.scalar.activation(
                        out=eT[:, st, :],
                        in_=pss,
                        func=mybir.ActivationFunctionType.Exp,
                        scale=SCALE,
                    )
                    if st == 0:
                        nc.vector.tensor_copy(acc_z, eT[:, 0, :])
                    else:
                        nc.vector.tensor_add(acc_z, acc_z, eT[:, st, :])
                # Z = ones^T @ acc_z16 -> [1, 512], transpose to columns via
                # tiny matmuls, reciprocal per 128-block
                acc_z16 = spool.tile([P, 512], F16, tag="acc_z16")
                nc.scalar.copy(acc_z16, acc_z)
                psz = psZ.tile([1, 512], F32, tag="psz")
                nc.tensor.matmul(
                    psz,
                    lhsT=ones_col[:, 0:1],
                    rhs=acc_z16,
                    start=True,
                    stop=True,
                )
                z16 = spool.tile([1, 512], F16, tag="z16")
                nc.scalar.copy(z16[0:1, :], psz[0:1, :])
                zcol = spool.tile([P, MT], F32, tag="zcol")
                for j in range(MT):
                    zc = psY.tile([P, 1], F32, tag="zc")
                    nc.tensor.matmul(
                        zc,
                        lhsT=z16[0:1, j * P : (j + 1) * P],
                        rhs=one_one[0:1, 0:1],
                        start=True,
                        stop=True,
                    )
                    nc.vector.reciprocal(zcol[:, j : j + 1], zc)
                # PV: out[q, k] tiles, q on partitions
                for j in range(MT):
                    pso = psO.tile([P, 512], F32, tag="pso")
                    for st in range(ST):
                        nc.tensor.matmul(
                            pso,
                            lhsT=eT[:, st, j * P : (j + 1) * P],
                            rhs=v_sb[:, st, :],
                            start=(st == 0),
                            stop=(st == ST - 1),
                        )
                    o = opool.tile([P, 512], F32, tag="o")
                    nc.scalar.activation(
                        out=o,
                        in_=pso,
                        func=mybir.ActivationFunctionType.Copy,
                        scale=zcol[:, j : j + 1],
                    )
                    ring = nc.sync if j % 2 == 0 else nc.scalar
                    ring.dma_start(
                        out=out[qc * 512 + j * P : qc * 512 + (j + 1) * P, :],
                        in_=o,
                    )

    if split_waits:
        _split_excess_waits(nc)
    return nc


_NC_CACHE = None


def _get_nc():
    global _NC_CACHE
    if _NC_CACHE is None:
        _NC_CACHE = build_nc()
    return _NC_CACHE


def _make_in_maps(x, Wq, bq, Wk, bk, Wv, bv):
    x = np.asarray(x, dtype=np.float32)
    wq16 = np.ascontiguousarray(np.asarray(Wq, np.float32).astype(np.float16))
    wk16 = np.ascontiguousarray(np.asarray(Wk, np.float32).astype(np.float16))
    wv16 = np.ascontiguousarray(np.asarray(Wv, np.float32).astype(np.float16))
    bq_c = np.ascontiguousarray(np.asarray(bq, np.float32).reshape(MT, P).T)
    bk_c = np.ascontiguousarray(np.asarray(bk, np.float32).reshape(MT, P).T)
    bv_b = np.ascontiguousarray(
        np.broadcast_to(
            np.asarray(bv, np.float32).astype(np.float16).reshape(1, DK),
            (P, DK),
        )
    )
    ones_c = np.ones((P, 1), dtype=np.float16)
    one1 = np.ones((1, 1), dtype=np.float16)
    in_maps = []
    for c in range(N_CORES):
        in_maps.append(
            {
                "xT": np.ascontiguousarray(x[c].T.astype(np.float16)),
                "wq": wq16,
                "wk": wk16,
                "wv": wv16,
                "bq": bq_c,
                "bk": bk_c,
                "bvb": bv_b,
                "ones_c": ones_c,
                "one1": one1,
            }
        )
    return in_maps


def run(x, Wq, bq, Wk, bk, Wv, bv, **run_kwargs):
    """Run on the 8 NeuronCores; returns (output, BassKernelResults)."""
    from concourse.bass_utils import run_bass_kernel_spmd

    nc = _get_nc()
    in_maps = _make_in_maps(x, Wq, bq, Wk, bk, Wv, bv)
    res = run_bass_kernel_spmd(
        nc, in_maps, core_ids=list(range(N_CORES)), **run_kwargs
    )
    out = np.stack([r["out"] for r in res.results], axis=0)
    return out, res


def kernel(x, Wq, bq, Wk, bk, Wv, bv):
    out, _ = run(x, Wq, bq, Wk, bk, Wv, bv)
    return out
